# revision 45
# baseline (speedup 1.0000x reference)
"""Multi-layer GATv2 on 8 Trainium2 NeuronCores (Bass/Tile).

Strategy (1D node partitioning per the sharding hint):
- Nodes split into 8 blocks of 12500; core m owns block m and all edges whose
  DESTINATION lies in its block (plus self-loops). Weights replicated.
- Per dst-group (128 nodes) the edge math runs on tiles of 128 edges:
    z[e,:]  = xl[src(e),:] + xr[dst(e),:]          (PSUM, via TensorE)
    alpha   = <att, leaky_relu(z)> per head        (ACT Prelu + DVE reduce)
    ea      = exp(alpha - 4)                       (constant bias; cancels)
    acc     = sum_e onehot_slot(e) * ea * [z | 1]  (one matmul per tile)
    out     = acc_z / acc_s - xr                   (all edges of a slot share
                                                    dst, so sum a*xl =
                                                    (sum ea*z)/S - xr)
  so the per-edge xl values are never re-gathered for the weighted sum.
- xr per edge comes from a one-hot (slot-major) matmul against the group's
  own 128 xr rows - no xr gather at all.
- Layer 0's xl[src] is staged on the host (x is an input): x[src] is uploaded
  pre-gathered in transposed per-edge tile layout and multiplied by Wl on
  device, so layer 0 issues NO dma_gather (the SWDGE descriptor generation on
  the Pool engine was the baseline bottleneck).
- Layer 1 computes xl=h1@Wl for all nodes (weights replicated, AllGather of
  h1^T between layers) and gathers per-edge rows with dma_gather in 4
  int16-range buckets, pipelined across groups so the Pool engine overlaps
  the rest of the machine.
- One activation table (exp/ln/prelu/identity) serves the whole kernel:
  leaky-relu is Prelu(alpha=0.2), rsqrt(v) = exp(-0.5*ln(v+eps)).
- Finalize (softmax division, LayerNorm, ELU, residual) is batched 4 dst
  groups at a time.
"""
import sys

sys.path.insert(0, "/opt/trn_rl_repo")

import numpy as np
import ml_dtypes

import concourse.bass as bass
import concourse.tile as tile
from concourse import bacc, mybir
from concourse.bass_utils import run_bass_kernel_spmd


# problem constants
N, D, H, L = 100000, 128, 4, 2
C = D // H
NEG_SLOPE = 0.2
LN_EPS = 1e-5

M = 8                # cores
NB = N // M          # 12500 nodes per block
NBP = 12544          # padded own-rows (98 * 128)
NT = NBP // 128      # 98 node tiles / groups per core
SRC_BUCKET = 25088   # M*NBP/4: equal buckets, int16 gather index range
NBUCK = 4
ALPHA_BIAS = 4.0     # subtracted inside exp (cancels in softmax ratio)

f8 = mybir.dt.float8e4
f16 = mybir.dt.float16
f32 = mybir.dt.float32
i16 = mybir.dt.int16
FP8 = ml_dtypes.float8_e4m3fn
FP16 = np.float16
FP32 = np.float32


# ---------------------------------------------------------------- host prep

def _wrap_idx(idx: np.ndarray) -> np.ndarray:
    """int16 index array -> dma_gather wrapped layout (128, n/16)."""
    n = idx.shape[0]
    assert n % 16 == 0
    a = idx.reshape(n // 16, 16).T.astype(np.int16)
    return np.tile(a, (8, 1))


def _onehots(slots: np.ndarray, t_tot: int):
    """slots: (t_tot*128,) int16 slot per edge position, -1 = pad.

    Returns (sT, s_t) fp8 arrays of shape (128, t_tot*128):
      sT : partition=slot, col=pos              (slot-major, lhsT for xr bcast)
      s_t: partition=e-in-tile, col=(t, slot)   (edge-major, lhsT for agg)
    """
    pos = np.arange(t_tot * 128)
    valid = slots >= 0
    sT = np.zeros((128, t_tot * 128), FP8)
    sT[slots[valid], pos[valid]] = 1.0
    s_t = np.zeros((t_tot * 128, 128), np.int8)
    s_t[pos[valid], slots[valid]] = 1
    s_t = s_t.reshape(t_tot, 128, 128).transpose(1, 0, 2).reshape(128, t_tot * 128)
    return sT, np.ascontiguousarray(s_t).astype(FP8)


def prep_edges(edge_index: np.ndarray, x16: np.ndarray):
    """Partition + sort + pad the edge list; build per-core staging arrays."""
    src = np.asarray(edge_index[0], np.int64)
    dst = np.asarray(edge_index[1], np.int64)
    loops = np.arange(N, dtype=np.int64)
    src = np.concatenate([src, loops])
    dst = np.concatenate([dst, loops])

    core_of = dst // NB
    dloc = dst - core_of * NB
    group = dloc // 128
    slot = dloc - group * 128

    out = {"cores": []}

    # ---------------- layer 0 layout: (core, group), no buckets
    order0 = np.lexsort((src, group, core_of))
    c0, g0 = core_of[order0], group[order0]
    s0, sl0 = src[order0], slot[order0]
    counts0 = np.zeros((M, NT), np.int64)
    np.add.at(counts0, (c0, g0), 1)
    tg0 = ((counts0.max(axis=0) + 127) // 128).astype(np.int64)   # (NT,)
    T0 = int(tg0.sum())
    tstart0 = np.concatenate([[0], np.cumsum(tg0)[:-1]])          # tiles
    starts0 = np.cumsum(counts0.reshape(-1)).reshape(M, NT) - counts0

    # ---------------- layer 1 layout: (core, group, bucket)
    # layer-1 xl lives in the AllGathered, block-PADDED layout
    # [M*NBP, 128]: global node n -> padded row (n//NB)*NBP + n%NB.
    # mod-4 interleaved buckets (gathered with elem_step=4 rows) spread
    # both random edges and self-loops evenly across the 4 runs per group
    psrc = (src // NB) * NBP + src % NB
    buck = psrc % NBUCK
    sloc = psrc // NBUCK
    order1 = np.lexsort((buck, group, core_of))
    c1, g1 = core_of[order1], group[order1]
    b1, sv1, sl1 = buck[order1], sloc[order1], slot[order1]
    counts1 = np.zeros((M, NT, NBUCK), np.int64)
    np.add.at(counts1, (c1, g1, b1), 1)
    maxcnt1 = counts1.max(axis=0)                                   # (NT, NBUCK)
    ktiles = ((maxcnt1 + 127) // 128).astype(np.int64)              # (NT, NBUCK)
    runs1 = ktiles.tolist()
    tg1 = ktiles.sum(axis=1)
    T1 = int(tg1.sum())
    rstart1 = np.zeros((NT, NBUCK), np.int64)
    acc = 0
    for g in range(NT):
        for b in range(NBUCK):
            rstart1[g, b] = acc
            acc += ktiles[g, b]
    assert acc == T1
    tstart1 = np.concatenate([[0], np.cumsum(tg1)[:-1]])
    starts1 = np.cumsum(counts1.reshape(-1)).reshape(M, NT, NBUCK) - counts1

    out.update(tg0=tg0.tolist(), T0=T0, tstart0=tstart0.tolist(),
               runs1=runs1, maxcnt1=maxcnt1.tolist(), tg1=tg1.tolist(), T1=T1,
               tstart1=tstart1.tolist(), rstart1=rstart1)

    xT = np.ascontiguousarray(x16.T)  # (128, N)

    for m in range(M):
        # layer 0 arrays
        slots0 = np.full(T0 * 128, -1, np.int16)
        esrc0 = np.full(T0 * 128, -1, np.int64)
        for g in range(NT):
            cnt = int(counts0[m, g])
            if cnt == 0:
                continue
            a = int(starts0[m, g])
            o = int(tstart0[g]) * 128
            slots0[o:o + cnt] = sl0[a:a + cnt]
            esrc0[o:o + cnt] = s0[a:a + cnt]
        sT0, s_t0 = _onehots(slots0, T0)
        xTsrc0 = np.zeros((128, T0 * 128), FP16)
        v = esrc0 >= 0
        xTsrc0[:, v] = xT[:, esrc0[v]]

        # layer 1 arrays
        slots1 = np.full(T1 * 128, -1, np.int16)
        gsrc1 = np.zeros(T1 * 128, np.int16)  # pad idx 0: finite data, onehot=0
        for g in range(NT):
            for b in range(NBUCK):
                cnt = int(counts1[m, g, b])
                if cnt == 0:
                    continue
                a = int(starts1[m, g, b])
                o = int(rstart1[g, b]) * 128
                slots1[o:o + cnt] = sl1[a:a + cnt]
                gsrc1[o:o + cnt] = sv1[a:a + cnt]
        sT1, s_t1 = _onehots(slots1, T1)

        out["cores"].append({
            "xTsrc0": xTsrc0,
            "sT0": sT0, "st0": s_t0,
            "sT1": sT1, "st1": s_t1,
            "gsrc1": _wrap_idx(gsrc1),
        })
    return out


# ------------------------------------------------------------- bass program

def _register_const_ap(nc, dtype, value):
    if (dtype, value) in nc.const_aps.aps:
        return
    t = nc.alloc_sbuf_tensor(f"const-{dtype.name}-{value}", [128, 1], dtype)
    nc.gpsimd.memset(t.ap(), value)
    nc.const_aps.aps[(dtype, value)] = t.ap()


def build(ep, affine=False, use_collective=True):
    """ep: dict from prep_edges (layouts only; per-core data via in_maps)."""
    nc = bacc.Bacc("TRN2", debug=False)
    _register_const_ap(nc, f32, -ALPHA_BIAS)
    _register_const_ap(nc, f32, LN_EPS)
    nc.all_engine_barrier()

    T0, T1 = ep["T0"], ep["T1"]
    tg = [ep["tg0"], ep["tg1"]]
    tstart = [ep["tstart0"], ep["tstart1"]]
    runs1 = ep["runs1"]
    maxcnt1 = ep["maxcnt1"]

    # ---- parameters (per-core values supplied via in_maps)
    xTsrc0_p = nc.declare_dram_parameter("xTsrc0", [128, T0 * 128], f16, isOutput=False)
    sT0_p = nc.declare_dram_parameter("sT0", [128, T0 * 128], f8, isOutput=False)
    st0_p = nc.declare_dram_parameter("st0", [128, T0 * 128], f8, isOutput=False)
    sT1_p = nc.declare_dram_parameter("sT1", [128, T1 * 128], f8, isOutput=False)
    st1_p = nc.declare_dram_parameter("st1", [128, T1 * 128], f8, isOutput=False)
    gsrc1_p = nc.declare_dram_parameter("gsrc1", [128, 8 * T1], i16, isOutput=False)
    xTown_p = nc.declare_dram_parameter("xTown", [128, NBP], f16, isOutput=False)
    xown_p = nc.declare_dram_parameter("xown", [NBP, 128], f16, isOutput=False)
    wl_p = nc.declare_dram_parameter("wl", [L, 128, 128], f16, isOutput=False)
    wr_p = nc.declare_dram_parameter("wr", [L, 128, 128], f16, isOutput=False)
    attB_p = nc.declare_dram_parameter("attB", [L, 128, 128], f16, isOutput=False)
    identh_p = nc.declare_dram_parameter("identh", [128, 128], f16, isOutput=False)
    if affine:
        biasB_p = nc.declare_dram_parameter("biasB", [L, 128, 128], f32, isOutput=False)
        gammaB_p = nc.declare_dram_parameter("gammaB", [L, 128, 128], f32, isOutput=False)
        betaB_p = nc.declare_dram_parameter("betaB", [L, 128, 128], f32, isOutput=False)
    hout = nc.declare_dram_parameter("hout", [NBP, 128], f32, isOutput=True)

    # ---- internal DRAM
    xl1own = nc.dram_tensor("xl1own", [NBP, 128], f16)
    xl_full = nc.dram_tensor("xl_full", [M * NBP, 128], f16, addr_space="Shared")
    h2own = nc.dram_tensor("h2own", [NBP, 128], f32)
    h2T_own = nc.dram_tensor("h2T_own", [128, NBP], f16)

    bucket_rows = [min(SRC_BUCKET, M * NBP - b * SRC_BUCKET) for b in range(NBUCK)]

    with tile.TileContext(nc) as tc:
        with (
            tc.tile_pool(name="const", bufs=1) as constp,
            tc.tile_pool(name="lconst", bufs=2) as lconstp,
            tc.tile_pool(name="xr", bufs=2) as xrp,
            tc.tile_pool(name="mm_in", bufs=3) as mm_in,
            tc.tile_pool(name="mm_ps", bufs=1, space="PSUM") as mm_ps,
            tc.tile_pool(name="mm_out", bufs=3) as mm_out,
            tc.tile_pool(name="edge", bufs=3) as edgep,
            tc.tile_pool(name="z_ps", bufs=4, space="PSUM") as zpool,
            tc.tile_pool(name="acc_ps", bufs=2, space="PSUM") as accp,
            tc.tile_pool(name="bt", bufs=3) as bp,
            tc.tile_pool(name="fin", bufs=2) as finp,
            tc.tile_pool(name="fin_ps", bufs=1, space="PSUM") as fin_ps,
        ):
            identh = constp.tile([128, 128], f16)
            nc.sync.dma_start(identh[:], identh_p[:])
            # preload all layer-1 gather indices once: no per-group idx waits
            gs_all = constp.tile([128, 8 * T1], i16)
            nc.sync.dma_start(gs_all[:], gsrc1_p[:])

            for layer in range(L):
                T = [T0, T1][layer]
                sT_p = [sT0_p, sT1_p][layer]
                st_p = [st0_p, st1_p][layer]

                wl_t = lconstp.tile([128, 128], f16, tag="wl")
                nc.sync.dma_start(wl_t[:], wl_p[layer])
                wr_t = lconstp.tile([128, 128], f16, tag="wr")
                nc.sync.dma_start(wr_t[:], wr_p[layer])
                attB_t = lconstp.tile([128, 128], f16, tag="attB")
                nc.sync.dma_start(attB_t[:], attB_p[layer])
                if affine:
                    biasB_t = lconstp.tile([128, 128], f32, tag="biasB")
                    nc.sync.dma_start(biasB_t[:], biasB_p[layer])
                    gammaB_t = lconstp.tile([128, 128], f32, tag="gammaB")
                    nc.sync.dma_start(gammaB_t[:], gammaB_p[layer])
                    betaB_t = lconstp.tile([128, 128], f32, tag="betaB")
                    nc.sync.dma_start(betaB_t[:], betaB_p[layer])

                # ---------------- xr for own nodes (kept in SBUF, node-major)
                xr_all = xrp.tile([128, NT, 128], f16, tag="xr")
                for q0 in range(0, NT, 4):
                    qn = min(4, NT - q0)
                    hT_t = mm_in.tile([128, 4 * 128], f16, tag="hT")
                    if layer == 0:
                        nc.sync.dma_start(hT_t[:, :qn * 128],
                                          xTown_p[:, q0 * 128:(q0 + qn) * 128])
                    else:
                        nc.sync.dma_start(hT_t[:, :qn * 128],
                                          h2T_own[:, q0 * 128:(q0 + qn) * 128])
                    for i in range(qn):
                        ps = mm_ps.tile([128, 128], f32, tag="mmps")
                        nc.tensor.matmul(ps[:], hT_t[:, i * 128:(i + 1) * 128],
                                         wr_t[:], start=True, stop=True)
                        nc.any.tensor_copy(xr_all[:, q0 + i, :], ps[:])

                # ---------------- edge + finalize, per dst group
                fb = None
                for g in range(NT):
                    tgg = int(tg[layer][g])
                    gt0 = int(tstart[layer][g])
                    sT_g = edgep.tile([128, tgg, 128], f8, tag="sT")
                    nc.sync.dma_start(sT_g[:], sT_p[:, gt0 * 128:(gt0 + tgg) * 128])
                    st_g = edgep.tile([128, tgg, 128], f8, tag="st")
                    nc.sync.dma_start(st_g[:], st_p[:, gt0 * 128:(gt0 + tgg) * 128])
                    if layer == 0:
                        xs_g = edgep.tile([128, tgg, 128], f16, tag="xs")
                        nc.sync.dma_start(
                            xs_g[:], xTsrc0_p[:, gt0 * 128:(gt0 + tgg) * 128])
                    else:
                        xl_e = edgep.tile([128, tgg, 128], f16, tag="xle")
                        # gathers are trimmed to the max-over-cores count, so
                        # run tails stay unwritten: zero them (0*inf guard)
                        nc.vector.memset(xl_e[:], 0.0)
                        xlf4 = xl_full[:].rearrange("(r f) c -> f r c", f=NBUCK)
                        r = 0
                        for b in range(NBUCK):
                            k = int(runs1[g][b])
                            if k == 0:
                                continue
                            for k0 in range(0, k, 8):
                                kk = min(8, k - k0)
                                o = 8 * (gt0 + r + k0)
                                nv = min(int(maxcnt1[g][b]) - k0 * 128, kk * 128)
                                nc.gpsimd.dma_gather(
                                    out_ap=xl_e[:, r + k0:r + k0 + kk, :],
                                    in_ap=xlf4[b],
                                    idxs_ap=gs_all[:, o:o + 8 * kk],
                                    num_idxs=nv,
                                    num_idxs_reg=nv,
                                    elem_size=128,
                                    elem_step=128 * NBUCK,
                                )
                            r += k

                    acc_g = accp.tile([128, 132], f32, tag="acc")
                    for q0 in range(0, tgg, 4):
                        qk = min(4, tgg - q0)
                        zps = zpool.tile([128, 4, 128], f32, tag="z")
                        for i in range(qk):
                            t = q0 + i
                            if layer == 0:
                                nc.tensor.matmul(zps[:, i, :], xs_g[:, t, :],
                                                 wl_t[:], start=True, stop=False)
                                nc.tensor.matmul(zps[:, i, :], sT_g[:, t, :],
                                                 xr_all[:, g, :],
                                                 start=False, stop=True)
                            else:
                                nc.tensor.matmul(zps[:, i, :], sT_g[:, t, :],
                                                 xr_all[:, g, :],
                                                 start=True, stop=False)
                                nc.tensor.matmul(zps[:, i, :], identh[:],
                                                 xl_e[:, t, :],
                                                 start=False, stop=True)
                        zl = bp.tile([128, 4, 128], f16, tag="zl")
                        nc.scalar.activation(zl[:, :qk, :], zps[:, :qk, :],
                                             mybir.ActivationFunctionType.Prelu,
                                             alpha=NEG_SLOPE)
                        tmp = bp.tile([128, 4, 128], f16, tag="tmp")
                        tmp_eng = nc.gpsimd if layer == 0 else nc.vector
                        tmp_eng.tensor_mul(
                            tmp[:, :qk, :], zl[:, :qk, :],
                            attB_t[:].unsqueeze(1).broadcast_to((128, qk, 128)))
                        al = bp.tile([128, 4, 4], f32, tag="al")
                        nc.vector.tensor_reduce(
                            al[:, :qk, :],
                            tmp[:, :qk, :].rearrange("p t (h c) -> p t h c", h=H),
                            axis=mybir.AxisListType.X,
                            op=mybir.AluOpType.add)
                        zw = bp.tile([128, 4, 132], f16, tag="zw")
                        nc.scalar.activation(zw[:, :qk, 128:132], al[:, :qk, :],
                                             mybir.ActivationFunctionType.Exp,
                                             bias=-ALPHA_BIAS)
                        nc.vector.tensor_mul(
                            zw[:, :qk, :128].rearrange("p t (h c) -> p t h c", h=H),
                            zps[:, :qk, :].rearrange("p t (h c) -> p t h c", h=H),
                            zw[:, :qk, 128:132].unsqueeze(3)
                            .broadcast_to((128, qk, H, C)))
                        for i in range(qk):
                            t = q0 + i
                            nc.tensor.matmul(acc_g[:], st_g[:, t, :], zw[:, i, :],
                                             start=(t == 0), stop=(t == tgg - 1))

                    if g % 4 == 0:
                        fb = finp.tile([128, 4, 132], f32, tag="fb")
                    nc.any.tensor_copy(fb[:, g % 4, :], acc_g[:])

                    # ---- finalize a batch of up to 4 groups
                    if g % 4 == 3 or g == NT - 1:
                        nb = g % 4 + 1
                        gb = g - nb + 1
                        nc.vector.tensor_scalar_add(
                            fb[:, :nb, 128:132], fb[:, :nb, 128:132], 1e-30)
                        rs = finp.tile([128, 4, 4], f32, tag="rs")
                        nc.vector.reciprocal(rs[:, :nb, :], fb[:, :nb, 128:132])
                        gv = finp.tile([128, 4, 128], f32, tag="gv")
                        nc.vector.tensor_mul(
                            gv[:, :nb, :].rearrange("p t (h c) -> p t h c", h=H),
                            fb[:, :nb, :128].rearrange("p t (h c) -> p t h c", h=H),
                            rs[:, :nb, :].unsqueeze(3).broadcast_to((128, nb, H, C)))
                        nc.vector.tensor_sub(gv[:, :nb, :], gv[:, :nb, :],
                                             xr_all[:, gb:gb + nb, :])
                        if affine:
                            nc.vector.tensor_add(
                                gv[:, :nb, :], gv[:, :nb, :],
                                biasB_t[:].unsqueeze(1).broadcast_to((128, nb, 128)))
                        bn6 = finp.tile([128, 4, 6], f32, tag="bn6")
                        bn2 = finp.tile([128, 4, 2], f32, tag="bn2")
                        for b in range(nb):
                            nc.vector.bn_stats(bn6[:, b, :], gv[:, b, :])
                            nc.vector.bn_aggr(bn2[:, b, :], bn6[:, b, :])
                        rstd = finp.tile([128, 4], f32, tag="rstd")
                        nc.scalar.activation(rstd[:, :nb], bn2[:, :nb, 1],
                                             mybir.ActivationFunctionType.Ln,
                                             bias=LN_EPS)
                        nc.scalar.activation(rstd[:, :nb], rstd[:, :nb],
                                             mybir.ActivationFunctionType.Exp,
                                             scale=-0.5)
                        nmr = finp.tile([128, 4], f32, tag="nmr")
                        nc.vector.scalar_tensor_tensor(
                            out=nmr[:, :nb], in0=bn2[:, :nb, 0], scalar=-1.0,
                            in1=rstd[:, :nb],
                            op0=mybir.AluOpType.mult, op1=mybir.AluOpType.mult)
                        yv = finp.tile([128, 4, 128], f32, tag="yv")
                        for b in range(nb):
                            nc.scalar.activation(
                                yv[:, b, :], gv[:, b, :],
                                mybir.ActivationFunctionType.Identity,
                                bias=nmr[:, b:b + 1], scale=rstd[:, b:b + 1])
                        if affine:
                            nc.vector.tensor_mul(
                                yv[:, :nb, :], yv[:, :nb, :],
                                gammaB_t[:].unsqueeze(1).broadcast_to((128, nb, 128)))
                            nc.vector.tensor_add(
                                yv[:, :nb, :], yv[:, :nb, :],
                                betaB_t[:].unsqueeze(1).broadcast_to((128, nb, 128)))
                        # elu(y) = exp(-relu(-y)) - 1 + relu(y), all on ACT
                        ym = finp.tile([128, 4, 128], f32, tag="ym")
                        nc.scalar.activation(ym[:, :nb, :], yv[:, :nb, :],
                                             mybir.ActivationFunctionType.Relu,
                                             scale=-1.0)
                        ee = finp.tile([128, 4, 128], f32, tag="ee")
                        nc.scalar.activation(ee[:, :nb, :], ym[:, :nb, :],
                                             mybir.ActivationFunctionType.Exp,
                                             scale=-1.0)
                        yx = finp.tile([128, 4, 128], f32, tag="yx")
                        nc.scalar.activation(yx[:, :nb, :], yv[:, :nb, :],
                                             mybir.ActivationFunctionType.Relu)
                        el = finp.tile([128, 4, 128], f32, tag="el")
                        nc.vector.scalar_tensor_tensor(
                            out=el[:, :nb, :], in0=ee[:, :nb, :], scalar=-1.0,
                            in1=yx[:, :nb, :],
                            op0=mybir.AluOpType.add, op1=mybir.AluOpType.add)
                        hp = finp.tile([128, 4, 128], f16 if layer == 0 else f32,
                                       tag=f"hp{layer}")
                        for b in range(nb):
                            if layer == 0:
                                nc.sync.dma_start(
                                    hp[:, b, :],
                                    xown_p[(gb + b) * 128:(gb + b + 1) * 128, :])
                            else:
                                nc.sync.dma_start(
                                    hp[:, b, :],
                                    h2own[(gb + b) * 128:(gb + b + 1) * 128, :])
                        hn = finp.tile([128, 4, 128], f32, tag="hn")
                        nc.vector.tensor_add(hn[:, :nb, :], hp[:, :nb, :],
                                             el[:, :nb, :])
                        if layer == 0:
                            h16 = finp.tile([128, 4, 128], f16, tag="h16")
                            nc.any.tensor_copy(h16[:, :nb, :], hn[:, :nb, :])
                            hT_sb = finp.tile([128, 4 * 128], f16, tag="htsb")
                            for b in range(nb):
                                nc.sync.dma_start(
                                    h2own[(gb + b) * 128:(gb + b + 1) * 128, :],
                                    hn[:, b, :])
                                hT_ps = fin_ps.tile([128, 128], f16, tag="finps")
                                nc.tensor.transpose(hT_ps[:], h16[:, b, :],
                                                    identh[:])
                                nc.any.tensor_copy(
                                    hT_sb[:, b * 128:(b + 1) * 128], hT_ps[:])
                            nc.sync.dma_start(
                                h2T_own[:, gb * 128:(gb + nb) * 128],
                                hT_sb[:, :nb * 128])
                        else:
                            for b in range(nb):
                                nc.sync.dma_start(
                                    hout[(gb + b) * 128:(gb + b + 1) * 128, :],
                                    hn[:, b, :])

                if layer == 0:
                    # xl1 for own nodes only (node-major), then AllGather the
                    # per-edge gather source - no all-blocks mm replication
                    wl1_t = lconstp.tile([128, 128], f16, tag="wl1")
                    nc.sync.dma_start(wl1_t[:], wl_p[1])
                    for q0 in range(0, NT, 4):
                        qn = min(4, NT - q0)
                        hT_t = mm_in.tile([128, 4 * 128], f16, tag="hT")
                        nc.sync.dma_start(hT_t[:, :qn * 128],
                                          h2T_own[:, q0 * 128:(q0 + qn) * 128])
                        ot = mm_out.tile([128, 4, 128], f16, tag="mmout")
                        for i in range(qn):
                            ps = mm_ps.tile([128, 128], f32, tag="mmps")
                            nc.tensor.matmul(ps[:], hT_t[:, i * 128:(i + 1) * 128],
                                             wl1_t[:], start=True, stop=True)
                            nc.any.tensor_copy(ot[:, i, :], ps[:])
                        nc.sync.dma_start(
                            xl1own[q0 * 128:(q0 + qn) * 128, :]
                            .rearrange("(i p) c -> p i c", p=128),
                            ot[:, :qn, :])
                    if use_collective:
                        nc.gpsimd.collective_compute(
                            "AllGather",
                            mybir.AluOpType.bypass,
                            replica_groups=[list(range(M))],
                            ins=[xl1own[:]],
                            outs=[xl_full[:]],
                        )
                    else:
                        for m in range(M):
                            nc.sync.dma_start(
                                xl_full[m * NBP:(m + 1) * NBP, :], xl1own[:])
    return nc


# ------------------------------------------------------------------ driver

def kernel(**inputs) -> np.ndarray:
    x = np.asarray(inputs["x"], FP32)
    edge_index = np.asarray(inputs["edge_index"])
    Wl = np.asarray(inputs["Wl"], FP32)
    Wr = np.asarray(inputs["Wr"], FP32)
    att = np.asarray(inputs["att"], FP32)
    bias = np.asarray(inputs["bias"], FP32)
    gamma = np.asarray(inputs["gamma"], FP32)
    beta = np.asarray(inputs["beta"], FP32)

    affine = not (np.all(bias == 0) and np.all(gamma == 1) and np.all(beta == 0))

    x16 = x.astype(FP16)
    ep = prep_edges(edge_index, x16)
    nc = build(ep, affine=affine,
               use_collective=bool(globals().get("USE_COLLECTIVE", True)))
    if not nc.is_finalized():
        nc.finalize()

    wl = Wl.astype(FP16)
    wr = Wr.astype(FP16)
    attB = np.broadcast_to(att.reshape(L, 1, H * C), (L, 128, H * C))
    identh = np.eye(128, dtype=FP16)

    in_maps = []
    for m in range(M):
        xo = np.zeros((NBP, 128), FP16)
        xo[:NB] = x16[m * NB:(m + 1) * NB]
        xoT = np.zeros((128, NBP), FP16)
        xoT[:, :NB] = x16[m * NB:(m + 1) * NB].T
        im = {
            "xTsrc0": ep["cores"][m]["xTsrc0"],
            "sT0": ep["cores"][m]["sT0"],
            "st0": ep["cores"][m]["st0"],
            "sT1": ep["cores"][m]["sT1"],
            "st1": ep["cores"][m]["st1"],
            "gsrc1": ep["cores"][m]["gsrc1"],
            "xTown": xoT,
            "xown": xo,
            "wl": wl, "wr": wr,
            "attB": np.ascontiguousarray(attB).astype(FP16),
            "identh": identh,
        }
        if affine:
            im["biasB"] = np.ascontiguousarray(
                np.broadcast_to(bias[:, None, :], (L, 128, 128))).astype(FP32)
            im["gammaB"] = np.ascontiguousarray(
                np.broadcast_to(gamma[:, None, :], (L, 128, 128))).astype(FP32)
            im["betaB"] = np.ascontiguousarray(
                np.broadcast_to(beta[:, None, :], (L, 128, 128))).astype(FP32)
        in_maps.append(im)

    res = run_bass_kernel_spmd(nc, in_maps, list(range(M)),
                               trace=bool(globals().get("TRACE", False)))
    global LAST_EXEC_NS
    LAST_EXEC_NS = res.exec_time_ns
    out = np.concatenate(
        [res.results[m]["hout"][:NB] for m in range(M)], axis=0)
    return out.astype(FP32)


if __name__ == "__main__":
    rng = np.random.default_rng(0)
    ei = rng.integers(0, N, (2, 1600000))
    x16 = rng.standard_normal((N, 128)).astype(FP16)
    ep = prep_edges(ei, x16)
    print(f"T0={ep['T0']} T1={ep['T1']} pad0={ep['T0']*128/(1700000/8):.3f} "
          f"pad1={ep['T1']*128/(1700000/8):.3f}")
    nc = build(ep)
    n_inst = sum(len(bb.instructions) for bb in nc.main_func.blocks)
    print(f"instructions: {n_inst}")


# revision 47
# speedup vs baseline: 1.1681x; 1.1681x over previous
"""Multi-layer GATv2 on 8 Trainium2 NeuronCores (Bass/Tile).

Strategy (1D node partitioning per the sharding hint):
- Nodes split into 8 blocks of 12500; core m owns block m and all edges whose
  DESTINATION lies in its block (plus self-loops). Weights replicated.
- Per dst-group (128 nodes) the edge math runs on tiles of 128 edges:
    z[e,:]  = xl[src(e),:] + xr[dst(e),:]          (PSUM, via TensorE)
    alpha   = <att, leaky_relu(z)> per head        (ACT Prelu + DVE reduce)
    ea      = exp(alpha - 4)                       (constant bias; cancels)
    acc     = sum_e onehot_slot(e) * ea * [z | 1]  (one matmul per tile)
    out     = acc_z / acc_s - xr                   (all edges of a slot share
                                                    dst, so sum a*xl =
                                                    (sum ea*z)/S - xr)
  so the per-edge xl values are never re-gathered for the weighted sum.
- xr per edge comes from a one-hot (slot-major) matmul against the group's
  own 128 xr rows - no xr gather at all.
- Layer 0's xl[src] is staged on the host (x is an input): x[src] is uploaded
  pre-gathered in transposed per-edge tile layout and multiplied by Wl on
  device, so layer 0 issues NO dma_gather (the SWDGE descriptor generation on
  the Pool engine was the baseline bottleneck).
- Layer 1 computes xl=h1@Wl for all nodes (weights replicated, AllGather of
  h1^T between layers) and gathers per-edge rows with dma_gather in 4
  int16-range buckets, pipelined across groups so the Pool engine overlaps
  the rest of the machine.
- One activation table (exp/ln/prelu/identity) serves the whole kernel:
  leaky-relu is Prelu(alpha=0.2), rsqrt(v) = exp(-0.5*ln(v+eps)).
- Finalize (softmax division, LayerNorm, ELU, residual) is batched 4 dst
  groups at a time.
"""
import sys

sys.path.insert(0, "/opt/trn_rl_repo")

import numpy as np
import ml_dtypes

import concourse.bass as bass
import concourse.tile as tile
from concourse import bacc, mybir
from concourse.bass_utils import run_bass_kernel_spmd


# problem constants
N, D, H, L = 100000, 128, 4, 2
C = D // H
NEG_SLOPE = 0.2
LN_EPS = 1e-5

M = 8                # cores
NB = N // M          # 12500 nodes per block
NBP = 12544          # padded own-rows (98 * 128)
NT = NBP // 128      # 98 node tiles / groups per core
SRC_BUCKET = 25088   # M*NBP/4: equal buckets, int16 gather index range
NBUCK = 4
ALPHA_BIAS = 4.0     # subtracted inside exp (cancels in softmax ratio)

f8 = mybir.dt.float8e4
f16 = mybir.dt.float16
f32 = mybir.dt.float32
i16 = mybir.dt.int16
FP8 = ml_dtypes.float8_e4m3fn
FP16 = np.float16
FP32 = np.float32


# ---------------------------------------------------------------- host prep

def _wrap_idx(idx: np.ndarray) -> np.ndarray:
    """int16 index array -> dma_gather wrapped layout (128, n/16)."""
    n = idx.shape[0]
    assert n % 16 == 0
    a = idx.reshape(n // 16, 16).T.astype(np.int16)
    return np.tile(a, (8, 1))


def _onehots(slots: np.ndarray, t_tot: int):
    """slots: (t_tot*128,) int16 slot per edge position, -1 = pad.

    Returns (sT, s_t) fp8 arrays of shape (128, t_tot*128):
      sT : partition=slot, col=pos              (slot-major, lhsT for xr bcast)
      s_t: partition=e-in-tile, col=(t, slot)   (edge-major, lhsT for agg)
    """
    pos = np.arange(t_tot * 128)
    valid = slots >= 0
    sT = np.zeros((128, t_tot * 128), FP8)
    sT[slots[valid], pos[valid]] = 1.0
    s_t = np.zeros((t_tot * 128, 128), np.int8)
    s_t[pos[valid], slots[valid]] = 1
    s_t = s_t.reshape(t_tot, 128, 128).transpose(1, 0, 2).reshape(128, t_tot * 128)
    return sT, np.ascontiguousarray(s_t).astype(FP8)


def prep_edges(edge_index: np.ndarray, x16: np.ndarray):
    """Partition + sort + pad the edge list; build per-core staging arrays."""
    src = np.asarray(edge_index[0], np.int64)
    dst = np.asarray(edge_index[1], np.int64)
    loops = np.arange(N, dtype=np.int64)
    src = np.concatenate([src, loops])
    dst = np.concatenate([dst, loops])

    core_of = dst // NB
    dloc = dst - core_of * NB
    group = dloc // 128
    slot = dloc - group * 128

    out = {"cores": []}

    # ---------------- layer 0 layout: (core, group), no buckets
    order0 = np.lexsort((src, group, core_of))
    c0, g0 = core_of[order0], group[order0]
    s0, sl0 = src[order0], slot[order0]
    counts0 = np.zeros((M, NT), np.int64)
    np.add.at(counts0, (c0, g0), 1)
    tg0 = ((counts0.max(axis=0) + 127) // 128).astype(np.int64)   # (NT,)
    T0 = int(tg0.sum())
    tstart0 = np.concatenate([[0], np.cumsum(tg0)[:-1]])          # tiles
    starts0 = np.cumsum(counts0.reshape(-1)).reshape(M, NT) - counts0

    # ---------------- layer 1 layout: (core, group, bucket)
    # layer-1 xl lives in the AllGathered, block-PADDED layout
    # [M*NBP, 128]: global node n -> padded row (n//NB)*NBP + n%NB.
    # mod-4 interleaved buckets (gathered with elem_step=4 rows) spread
    # both random edges and self-loops evenly across the 4 runs per group
    psrc = (src // NB) * NBP + src % NB
    buck = psrc % NBUCK
    sloc = psrc // NBUCK
    order1 = np.lexsort((buck, group, core_of))
    c1, g1 = core_of[order1], group[order1]
    b1, sv1, sl1 = buck[order1], sloc[order1], slot[order1]
    counts1 = np.zeros((M, NT, NBUCK), np.int64)
    np.add.at(counts1, (c1, g1, b1), 1)
    ktiles = ((counts1.max(axis=0) + 127) // 128).astype(np.int64)  # (NT, NBUCK)
    runs1 = ktiles.tolist()
    tg1 = ktiles.sum(axis=1)
    T1 = int(tg1.sum())
    rstart1 = np.zeros((NT, NBUCK), np.int64)
    acc = 0
    for g in range(NT):
        for b in range(NBUCK):
            rstart1[g, b] = acc
            acc += ktiles[g, b]
    assert acc == T1
    tstart1 = np.concatenate([[0], np.cumsum(tg1)[:-1]])
    starts1 = np.cumsum(counts1.reshape(-1)).reshape(M, NT, NBUCK) - counts1

    out.update(tg0=tg0.tolist(), T0=T0, tstart0=tstart0.tolist(),
               runs1=runs1, tg1=tg1.tolist(), T1=T1,
               tstart1=tstart1.tolist(), rstart1=rstart1)

    xT = np.ascontiguousarray(x16.T)  # (128, N)

    for m in range(M):
        # layer 0 arrays
        slots0 = np.full(T0 * 128, -1, np.int16)
        esrc0 = np.full(T0 * 128, -1, np.int64)
        for g in range(NT):
            cnt = int(counts0[m, g])
            if cnt == 0:
                continue
            a = int(starts0[m, g])
            o = int(tstart0[g]) * 128
            slots0[o:o + cnt] = sl0[a:a + cnt]
            esrc0[o:o + cnt] = s0[a:a + cnt]
        sT0, s_t0 = _onehots(slots0, T0)
        xTsrc0 = np.zeros((128, T0 * 128), FP16)
        v = esrc0 >= 0
        xTsrc0[:, v] = xT[:, esrc0[v]]

        # layer 1 arrays
        slots1 = np.full(T1 * 128, -1, np.int16)
        gsrc1 = np.zeros(T1 * 128, np.int16)  # pad idx 0: finite data, onehot=0
        for g in range(NT):
            for b in range(NBUCK):
                cnt = int(counts1[m, g, b])
                if cnt == 0:
                    continue
                a = int(starts1[m, g, b])
                o = int(rstart1[g, b]) * 128
                slots1[o:o + cnt] = sl1[a:a + cnt]
                gsrc1[o:o + cnt] = sv1[a:a + cnt]
        sT1, s_t1 = _onehots(slots1, T1)

        out["cores"].append({
            "xTsrc0": xTsrc0,
            "sT0": sT0, "st0": s_t0,
            "sT1": sT1, "st1": s_t1,
            "gsrc1": _wrap_idx(gsrc1),
        })
    return out


# ------------------------------------------------------------- bass program

def _register_const_ap(nc, dtype, value):
    if (dtype, value) in nc.const_aps.aps:
        return
    t = nc.alloc_sbuf_tensor(f"const-{dtype.name}-{value}", [128, 1], dtype)
    nc.gpsimd.memset(t.ap(), value)
    nc.const_aps.aps[(dtype, value)] = t.ap()


def build(ep, affine=False, use_collective=True):
    """ep: dict from prep_edges (layouts only; per-core data via in_maps)."""
    nc = bacc.Bacc("TRN2", debug=False)
    _register_const_ap(nc, f32, -ALPHA_BIAS)
    _register_const_ap(nc, f32, LN_EPS)
    nc.all_engine_barrier()

    T0, T1 = ep["T0"], ep["T1"]
    tg = [ep["tg0"], ep["tg1"]]
    tstart = [ep["tstart0"], ep["tstart1"]]
    runs1 = ep["runs1"]

    # ---- parameters (per-core values supplied via in_maps)
    xTsrc0_p = nc.declare_dram_parameter("xTsrc0", [128, T0 * 128], f16, isOutput=False)
    sT0_p = nc.declare_dram_parameter("sT0", [128, T0 * 128], f8, isOutput=False)
    st0_p = nc.declare_dram_parameter("st0", [128, T0 * 128], f8, isOutput=False)
    sT1_p = nc.declare_dram_parameter("sT1", [128, T1 * 128], f8, isOutput=False)
    st1_p = nc.declare_dram_parameter("st1", [128, T1 * 128], f8, isOutput=False)
    gsrc1_p = nc.declare_dram_parameter("gsrc1", [128, 8 * T1], i16, isOutput=False)
    xTown_p = nc.declare_dram_parameter("xTown", [128, NBP], f16, isOutput=False)
    xown_p = nc.declare_dram_parameter("xown", [NBP, 128], f16, isOutput=False)
    wl_p = nc.declare_dram_parameter("wl", [L, 128, 128], f16, isOutput=False)
    wr_p = nc.declare_dram_parameter("wr", [L, 128, 128], f16, isOutput=False)
    attB_p = nc.declare_dram_parameter("attB", [L, 128, 128], f16, isOutput=False)
    identh_p = nc.declare_dram_parameter("identh", [128, 128], f16, isOutput=False)
    if affine:
        biasB_p = nc.declare_dram_parameter("biasB", [L, 128, 128], f32, isOutput=False)
        gammaB_p = nc.declare_dram_parameter("gammaB", [L, 128, 128], f32, isOutput=False)
        betaB_p = nc.declare_dram_parameter("betaB", [L, 128, 128], f32, isOutput=False)
    hout = nc.declare_dram_parameter("hout", [NBP, 128], f32, isOutput=True)

    # ---- internal DRAM
    xl1own = nc.dram_tensor("xl1own", [NBP, 128], f16)
    xl_full = nc.dram_tensor("xl_full", [M * NBP, 128], f16, addr_space="Shared")
    h2own = nc.dram_tensor("h2own", [NBP, 128], f32)
    h2T_own = nc.dram_tensor("h2T_own", [128, NBP], f16)

    bucket_rows = [min(SRC_BUCKET, M * NBP - b * SRC_BUCKET) for b in range(NBUCK)]

    with tile.TileContext(nc) as tc:
        with (
            tc.tile_pool(name="const", bufs=1) as constp,
            tc.tile_pool(name="lconst", bufs=2) as lconstp,
            tc.tile_pool(name="xr", bufs=2) as xrp,
            tc.tile_pool(name="mm_in", bufs=3) as mm_in,
            tc.tile_pool(name="mm_ps", bufs=1, space="PSUM") as mm_ps,
            tc.tile_pool(name="mm_out", bufs=3) as mm_out,
            tc.tile_pool(name="edge", bufs=3) as edgep,
            tc.tile_pool(name="z_ps", bufs=4, space="PSUM") as zpool,
            tc.tile_pool(name="acc_ps", bufs=2, space="PSUM") as accp,
            tc.tile_pool(name="bt", bufs=3) as bp,
            tc.tile_pool(name="fin", bufs=2) as finp,
            tc.tile_pool(name="fin_ps", bufs=1, space="PSUM") as fin_ps,
        ):
            identh = constp.tile([128, 128], f16)
            nc.sync.dma_start(identh[:], identh_p[:])
            # preload all layer-1 gather indices once: no per-group idx waits
            gs_all = constp.tile([128, 8 * T1], i16)
            nc.sync.dma_start(gs_all[:], gsrc1_p[:])

            for layer in range(L):
                T = [T0, T1][layer]
                sT_p = [sT0_p, sT1_p][layer]
                st_p = [st0_p, st1_p][layer]

                wl_t = lconstp.tile([128, 128], f16, tag="wl")
                nc.sync.dma_start(wl_t[:], wl_p[layer])
                wr_t = lconstp.tile([128, 128], f16, tag="wr")
                nc.sync.dma_start(wr_t[:], wr_p[layer])
                attB_t = lconstp.tile([128, 128], f16, tag="attB")
                nc.sync.dma_start(attB_t[:], attB_p[layer])
                if affine:
                    biasB_t = lconstp.tile([128, 128], f32, tag="biasB")
                    nc.sync.dma_start(biasB_t[:], biasB_p[layer])
                    gammaB_t = lconstp.tile([128, 128], f32, tag="gammaB")
                    nc.sync.dma_start(gammaB_t[:], gammaB_p[layer])
                    betaB_t = lconstp.tile([128, 128], f32, tag="betaB")
                    nc.sync.dma_start(betaB_t[:], betaB_p[layer])

                # ---------------- xr for own nodes (kept in SBUF, node-major)
                xr_all = xrp.tile([128, NT, 128], f16, tag="xr")
                for q0 in range(0, NT, 4):
                    qn = min(4, NT - q0)
                    hT_t = mm_in.tile([128, 4 * 128], f16, tag="hT")
                    if layer == 0:
                        nc.sync.dma_start(hT_t[:, :qn * 128],
                                          xTown_p[:, q0 * 128:(q0 + qn) * 128])
                    else:
                        nc.sync.dma_start(hT_t[:, :qn * 128],
                                          h2T_own[:, q0 * 128:(q0 + qn) * 128])
                    for i in range(qn):
                        ps = mm_ps.tile([128, 128], f32, tag="mmps")
                        nc.tensor.matmul(ps[:], hT_t[:, i * 128:(i + 1) * 128],
                                         wr_t[:], start=True, stop=True)
                        nc.any.tensor_copy(xr_all[:, q0 + i, :], ps[:])

                # ---------------- edge + finalize, per dst group
                fb = None
                for g in range(NT):
                    tgg = int(tg[layer][g])
                    gt0 = int(tstart[layer][g])
                    sT_g = edgep.tile([128, tgg, 128], f8, tag="sT")
                    nc.sync.dma_start(sT_g[:], sT_p[:, gt0 * 128:(gt0 + tgg) * 128])
                    st_g = edgep.tile([128, tgg, 128], f8, tag="st")
                    nc.sync.dma_start(st_g[:], st_p[:, gt0 * 128:(gt0 + tgg) * 128])
                    if layer == 0:
                        xs_g = edgep.tile([128, tgg, 128], f16, tag="xs")
                        nc.sync.dma_start(
                            xs_g[:], xTsrc0_p[:, gt0 * 128:(gt0 + tgg) * 128])
                    else:
                        xl_e = edgep.tile([128, tgg, 128], f16, tag="xle")
                        xlf4 = xl_full[:].rearrange("(r f) c -> f r c", f=NBUCK)
                        r = 0
                        for b in range(NBUCK):
                            k = int(runs1[g][b])
                            if k == 0:
                                continue
                            for k0 in range(0, k, 8):
                                kk = min(8, k - k0)
                                o = 8 * (gt0 + r + k0)
                                nc.gpsimd.dma_gather(
                                    out_ap=xl_e[:, r + k0:r + k0 + kk, :],
                                    in_ap=xlf4[b],
                                    idxs_ap=gs_all[:, o:o + 8 * kk],
                                    num_idxs=kk * 128,
                                    num_idxs_reg=kk * 128,
                                    elem_size=128,
                                    elem_step=128 * NBUCK,
                                )
                            r += k

                    acc_g = accp.tile([128, 132], f32, tag="acc")
                    for q0 in range(0, tgg, 4):
                        qk = min(4, tgg - q0)
                        zps = zpool.tile([128, 4, 128], f32, tag="z")
                        for i in range(qk):
                            t = q0 + i
                            if layer == 0:
                                nc.tensor.matmul(zps[:, i, :], xs_g[:, t, :],
                                                 wl_t[:], start=True, stop=False)
                                nc.tensor.matmul(zps[:, i, :], sT_g[:, t, :],
                                                 xr_all[:, g, :],
                                                 start=False, stop=True)
                            else:
                                nc.tensor.matmul(zps[:, i, :], sT_g[:, t, :],
                                                 xr_all[:, g, :],
                                                 start=True, stop=False)
                                nc.tensor.matmul(zps[:, i, :], identh[:],
                                                 xl_e[:, t, :],
                                                 start=False, stop=True)
                        zl = bp.tile([128, 4, 128], f16, tag="zl")
                        nc.scalar.activation(zl[:, :qk, :], zps[:, :qk, :],
                                             mybir.ActivationFunctionType.Prelu,
                                             alpha=NEG_SLOPE)
                        tmp = bp.tile([128, 4, 128], f16, tag="tmp")
                        tmp_eng = nc.gpsimd if layer == 0 else nc.vector
                        tmp_eng.tensor_mul(
                            tmp[:, :qk, :], zl[:, :qk, :],
                            attB_t[:].unsqueeze(1).broadcast_to((128, qk, 128)))
                        al = bp.tile([128, 4, 4], f32, tag="al")
                        nc.vector.tensor_reduce(
                            al[:, :qk, :],
                            tmp[:, :qk, :].rearrange("p t (h c) -> p t h c", h=H),
                            axis=mybir.AxisListType.X,
                            op=mybir.AluOpType.add)
                        zw = bp.tile([128, 4, 132], f16, tag="zw")
                        nc.scalar.activation(zw[:, :qk, 128:132], al[:, :qk, :],
                                             mybir.ActivationFunctionType.Exp,
                                             bias=-ALPHA_BIAS)
                        nc.vector.tensor_mul(
                            zw[:, :qk, :128].rearrange("p t (h c) -> p t h c", h=H),
                            zps[:, :qk, :].rearrange("p t (h c) -> p t h c", h=H),
                            zw[:, :qk, 128:132].unsqueeze(3)
                            .broadcast_to((128, qk, H, C)))
                        for i in range(qk):
                            t = q0 + i
                            nc.tensor.matmul(acc_g[:], st_g[:, t, :], zw[:, i, :],
                                             start=(t == 0), stop=(t == tgg - 1))

                    if g % 4 == 0:
                        fb = finp.tile([128, 4, 132], f32, tag="fb")
                    nc.any.tensor_copy(fb[:, g % 4, :], acc_g[:])

                    # ---- finalize a batch of up to 4 groups
                    if g % 4 == 3 or g == NT - 1:
                        nb = g % 4 + 1
                        gb = g - nb + 1
                        nc.vector.tensor_scalar_add(
                            fb[:, :nb, 128:132], fb[:, :nb, 128:132], 1e-30)
                        rs = finp.tile([128, 4, 4], f32, tag="rs")
                        nc.vector.reciprocal(rs[:, :nb, :], fb[:, :nb, 128:132])
                        gv = finp.tile([128, 4, 128], f32, tag="gv")
                        nc.vector.tensor_mul(
                            gv[:, :nb, :].rearrange("p t (h c) -> p t h c", h=H),
                            fb[:, :nb, :128].rearrange("p t (h c) -> p t h c", h=H),
                            rs[:, :nb, :].unsqueeze(3).broadcast_to((128, nb, H, C)))
                        nc.vector.tensor_sub(gv[:, :nb, :], gv[:, :nb, :],
                                             xr_all[:, gb:gb + nb, :])
                        if affine:
                            nc.vector.tensor_add(
                                gv[:, :nb, :], gv[:, :nb, :],
                                biasB_t[:].unsqueeze(1).broadcast_to((128, nb, 128)))
                        bn6 = finp.tile([128, 4, 6], f32, tag="bn6")
                        bn2 = finp.tile([128, 4, 2], f32, tag="bn2")
                        for b in range(nb):
                            nc.vector.bn_stats(bn6[:, b, :], gv[:, b, :])
                            nc.vector.bn_aggr(bn2[:, b, :], bn6[:, b, :])
                        rstd = finp.tile([128, 4], f32, tag="rstd")
                        nc.scalar.activation(rstd[:, :nb], bn2[:, :nb, 1],
                                             mybir.ActivationFunctionType.Ln,
                                             bias=LN_EPS)
                        nc.scalar.activation(rstd[:, :nb], rstd[:, :nb],
                                             mybir.ActivationFunctionType.Exp,
                                             scale=-0.5)
                        nmr = finp.tile([128, 4], f32, tag="nmr")
                        nc.vector.scalar_tensor_tensor(
                            out=nmr[:, :nb], in0=bn2[:, :nb, 0], scalar=-1.0,
                            in1=rstd[:, :nb],
                            op0=mybir.AluOpType.mult, op1=mybir.AluOpType.mult)
                        yv = finp.tile([128, 4, 128], f32, tag="yv")
                        for b in range(nb):
                            nc.scalar.activation(
                                yv[:, b, :], gv[:, b, :],
                                mybir.ActivationFunctionType.Identity,
                                bias=nmr[:, b:b + 1], scale=rstd[:, b:b + 1])
                        if affine:
                            nc.vector.tensor_mul(
                                yv[:, :nb, :], yv[:, :nb, :],
                                gammaB_t[:].unsqueeze(1).broadcast_to((128, nb, 128)))
                            nc.vector.tensor_add(
                                yv[:, :nb, :], yv[:, :nb, :],
                                betaB_t[:].unsqueeze(1).broadcast_to((128, nb, 128)))
                        # elu(y) = exp(-relu(-y)) - 1 + relu(y), all on ACT
                        ym = finp.tile([128, 4, 128], f32, tag="ym")
                        nc.scalar.activation(ym[:, :nb, :], yv[:, :nb, :],
                                             mybir.ActivationFunctionType.Relu,
                                             scale=-1.0)
                        ee = finp.tile([128, 4, 128], f32, tag="ee")
                        nc.scalar.activation(ee[:, :nb, :], ym[:, :nb, :],
                                             mybir.ActivationFunctionType.Exp,
                                             scale=-1.0)
                        yx = finp.tile([128, 4, 128], f32, tag="yx")
                        nc.scalar.activation(yx[:, :nb, :], yv[:, :nb, :],
                                             mybir.ActivationFunctionType.Relu)
                        el = finp.tile([128, 4, 128], f32, tag="el")
                        nc.vector.scalar_tensor_tensor(
                            out=el[:, :nb, :], in0=ee[:, :nb, :], scalar=-1.0,
                            in1=yx[:, :nb, :],
                            op0=mybir.AluOpType.add, op1=mybir.AluOpType.add)
                        hp = finp.tile([128, 4, 128], f16 if layer == 0 else f32,
                                       tag=f"hp{layer}")
                        for b in range(nb):
                            if layer == 0:
                                nc.sync.dma_start(
                                    hp[:, b, :],
                                    xown_p[(gb + b) * 128:(gb + b + 1) * 128, :])
                            else:
                                nc.sync.dma_start(
                                    hp[:, b, :],
                                    h2own[(gb + b) * 128:(gb + b + 1) * 128, :])
                        hn = finp.tile([128, 4, 128], f32, tag="hn")
                        nc.vector.tensor_add(hn[:, :nb, :], hp[:, :nb, :],
                                             el[:, :nb, :])
                        if layer == 0:
                            h16 = finp.tile([128, 4, 128], f16, tag="h16")
                            nc.any.tensor_copy(h16[:, :nb, :], hn[:, :nb, :])
                            hT_sb = finp.tile([128, 4 * 128], f16, tag="htsb")
                            for b in range(nb):
                                nc.sync.dma_start(
                                    h2own[(gb + b) * 128:(gb + b + 1) * 128, :],
                                    hn[:, b, :])
                                hT_ps = fin_ps.tile([128, 128], f16, tag="finps")
                                nc.tensor.transpose(hT_ps[:], h16[:, b, :],
                                                    identh[:])
                                nc.any.tensor_copy(
                                    hT_sb[:, b * 128:(b + 1) * 128], hT_ps[:])
                            nc.sync.dma_start(
                                h2T_own[:, gb * 128:(gb + nb) * 128],
                                hT_sb[:, :nb * 128])
                        else:
                            for b in range(nb):
                                nc.sync.dma_start(
                                    hout[(gb + b) * 128:(gb + b + 1) * 128, :],
                                    hn[:, b, :])

                if layer == 0:
                    # xl1 for own nodes only (node-major), then AllGather the
                    # per-edge gather source - no all-blocks mm replication
                    wl1_t = lconstp.tile([128, 128], f16, tag="wl1")
                    nc.sync.dma_start(wl1_t[:], wl_p[1])
                    for q0 in range(0, NT, 4):
                        qn = min(4, NT - q0)
                        hT_t = mm_in.tile([128, 4 * 128], f16, tag="hT")
                        nc.sync.dma_start(hT_t[:, :qn * 128],
                                          h2T_own[:, q0 * 128:(q0 + qn) * 128])
                        ot = mm_out.tile([128, 4, 128], f16, tag="mmout")
                        for i in range(qn):
                            ps = mm_ps.tile([128, 128], f32, tag="mmps")
                            nc.tensor.matmul(ps[:], hT_t[:, i * 128:(i + 1) * 128],
                                             wl1_t[:], start=True, stop=True)
                            nc.any.tensor_copy(ot[:, i, :], ps[:])
                        nc.sync.dma_start(
                            xl1own[q0 * 128:(q0 + qn) * 128, :]
                            .rearrange("(i p) c -> p i c", p=128),
                            ot[:, :qn, :])
                    if use_collective:
                        nc.gpsimd.collective_compute(
                            "AllGather",
                            mybir.AluOpType.bypass,
                            replica_groups=[list(range(M))],
                            ins=[xl1own[:]],
                            outs=[xl_full[:]],
                        )
                    else:
                        for m in range(M):
                            nc.sync.dma_start(
                                xl_full[m * NBP:(m + 1) * NBP, :], xl1own[:])
    return nc


# ------------------------------------------------------------------ driver

def kernel(**inputs) -> np.ndarray:
    x = np.asarray(inputs["x"], FP32)
    edge_index = np.asarray(inputs["edge_index"])
    Wl = np.asarray(inputs["Wl"], FP32)
    Wr = np.asarray(inputs["Wr"], FP32)
    att = np.asarray(inputs["att"], FP32)
    bias = np.asarray(inputs["bias"], FP32)
    gamma = np.asarray(inputs["gamma"], FP32)
    beta = np.asarray(inputs["beta"], FP32)

    affine = not (np.all(bias == 0) and np.all(gamma == 1) and np.all(beta == 0))

    x16 = x.astype(FP16)
    ep = prep_edges(edge_index, x16)
    nc = build(ep, affine=affine,
               use_collective=bool(globals().get("USE_COLLECTIVE", True)))
    if not nc.is_finalized():
        nc.finalize()

    wl = Wl.astype(FP16)
    wr = Wr.astype(FP16)
    attB = np.broadcast_to(att.reshape(L, 1, H * C), (L, 128, H * C))
    identh = np.eye(128, dtype=FP16)

    in_maps = []
    for m in range(M):
        xo = np.zeros((NBP, 128), FP16)
        xo[:NB] = x16[m * NB:(m + 1) * NB]
        xoT = np.zeros((128, NBP), FP16)
        xoT[:, :NB] = x16[m * NB:(m + 1) * NB].T
        im = {
            "xTsrc0": ep["cores"][m]["xTsrc0"],
            "sT0": ep["cores"][m]["sT0"],
            "st0": ep["cores"][m]["st0"],
            "sT1": ep["cores"][m]["sT1"],
            "st1": ep["cores"][m]["st1"],
            "gsrc1": ep["cores"][m]["gsrc1"],
            "xTown": xoT,
            "xown": xo,
            "wl": wl, "wr": wr,
            "attB": np.ascontiguousarray(attB).astype(FP16),
            "identh": identh,
        }
        if affine:
            im["biasB"] = np.ascontiguousarray(
                np.broadcast_to(bias[:, None, :], (L, 128, 128))).astype(FP32)
            im["gammaB"] = np.ascontiguousarray(
                np.broadcast_to(gamma[:, None, :], (L, 128, 128))).astype(FP32)
            im["betaB"] = np.ascontiguousarray(
                np.broadcast_to(beta[:, None, :], (L, 128, 128))).astype(FP32)
        in_maps.append(im)

    res = run_bass_kernel_spmd(nc, in_maps, list(range(M)),
                               trace=bool(globals().get("TRACE", False)))
    global LAST_EXEC_NS
    LAST_EXEC_NS = res.exec_time_ns
    out = np.concatenate(
        [res.results[m]["hout"][:NB] for m in range(M)], axis=0)
    return out.astype(FP32)


if __name__ == "__main__":
    rng = np.random.default_rng(0)
    ei = rng.integers(0, N, (2, 1600000))
    x16 = rng.standard_normal((N, 128)).astype(FP16)
    ep = prep_edges(ei, x16)
    print(f"T0={ep['T0']} T1={ep['T1']} pad0={ep['T0']*128/(1700000/8):.3f} "
          f"pad1={ep['T1']*128/(1700000/8):.3f}")
    nc = build(ep)
    n_inst = sum(len(bb.instructions) for bb in nc.main_func.blocks)
    print(f"instructions: {n_inst}")
